# revision 23
# baseline (speedup 1.0000x reference)
"""Trainium2 Bass kernel for nn_CrossAttentionBlock.

Math: with key/value seq_len == 1 the attention softmax is identically 1, so
q/k (and masked_x entirely) never affect the output:

    out[n, :] = LN(((graph_vec @ Wv.T + bv) @ Wiv.T + biv) @ Wout.T + bout)[batch_indices[n]]

i.e. a 128-row lookup table indexed by batch_indices. Strategy per core
(data-parallel over nodes, 8 cores x 50000 nodes):

  1. prologue: compute the [128, 128] table on-device from host-transposed
     weight layouts (3 matmuls + bias rank-1 updates + bn_stats LayerNorm),
     cast to fp16 (rel err ~5e-4, far under the 2e-2 gate).
  2. main loop per 512-node group (one PSUM bank):
       - idx chunk DMA (1 KB fp8) onto partition 0, ring of 12, Pool-queue
         trigger (a single [1, N] idx load would serialize ~38 us on one
         SBUF partition's write port)
       - broadcast idx across partitions with ONE PE DoubleRow fp8 matmul:
         idx = 16q + r split host-side, both fp8-exact; ones (x) [q; r]
         accumulates q+r = idx into PSUM at 0.5 cycles/column
       - DVE is_equal against a partition-iota column -> one-hot^T (fp16)
       - 4x PE fp16 matmuls: out[node, h] = onehotT.T @ tbl
       - PSUM -> SBUF staging copy (Scalar; every 6th on DVE)
       - one 512 KiB DMA store per 2 groups

The only irreducible HBM traffic is the 25.7 MiB/core fp32 output write
(~75 us at ~350 GB/s); every other engine is scheduled to stay under that.
"""

import sys

if "/opt/trn_rl_repo" not in sys.path:
    sys.path.insert(0, "/opt/trn_rl_repo")

import numpy as np

import concourse.bass as bass
import concourse.bacc as bacc
import concourse.tile as tile
from concourse import mybir
from concourse import bass_utils

F32 = mybir.dt.float32
F16 = mybir.dt.float16
F8 = mybir.dt.float8e4
DR = mybir.MatmulPerfMode.DoubleRow

N_NODES = 400000
H = 128          # hidden
G = 256          # graph_dim
B = 128          # batch (table rows)
N_CORES = 8
NSHARD = N_NODES // N_CORES          # 50000
GROUP = 512                          # nodes per group (one PSUM bank)
NPAD = 50176                         # 98 * 512, per-core padded shard
NGROUPS = NPAD // GROUP              # 98
STORE_G = 4                          # groups per DMA store (1 MiB)
EPS = 1e-5

# Schedule knobs (variant tuple: (dr_bcast, copy_sched, pool_mod))
#   dr_bcast: broadcast via one fp8 DoubleRow matmul (else fp16 K=1 matmul)
#   copy_sched: string over {"A","V"} cycled per group for the PSUM->stage
#               copy engine (Scalar/DVE; GpSimd cannot read PSUM)
#   pool_mod: m -> groups with g % m == m-1 broadcast on GpSimd
#             partition_broadcast (None = all broadcasts on PE)
DEFAULT_VARIANT = (False, "AAAAAAAV", 2)


def _row1(ap):
    """View a 1-D DRAM AP as [1, N]."""
    return bass.AP(tensor=ap.tensor, offset=ap.offset, ap=[[0, 1]] + list(ap.ap))


def _bcast128(ap):
    """View a 1-D DRAM AP as [128, N] replicated across partitions."""
    return bass.AP(tensor=ap.tensor, offset=ap.offset, ap=[[0, 128]] + list(ap.ap))


def build_bass(variant=DEFAULT_VARIANT):
    dr_bcast, copy_sched, pool_mod = variant
    nc = bacc.Bacc("TRN2", target_bir_lowering=False)

    # packed host-transposed weights (two DMAs):
    # fp16: [gvT0|gvT1|WvT0|WvT1|WivT|WoutT|bout_row@p0]
    # fp32: [bv_col|biv_col|gamma replicated|beta replicated]
    wb_d = nc.dram_tensor("wblob", [128, 7 * 128], F16, kind="ExternalInput")
    mb_d = nc.dram_tensor("mblob", [128, 2 + 2 * 128], F32, kind="ExternalInput")
    idx_d = nc.dram_tensor("idx", [NGROUPS, GROUP], F16, kind="ExternalInput")
    out_d = nc.dram_tensor("out", [NPAD, H], F32, kind="ExternalOutput")

    with tile.TileContext(nc) as tc:
        with (
            tc.tile_pool(name="singles", bufs=1) as singles,
            tc.tile_pool(name="oh", bufs=6) as oh_pool,
            tc.tile_pool(name="bcsb", bufs=3) as bcsb_pool,
            tc.tile_pool(name="bps", bufs=3, space="PSUM") as bc_ps_pool,
            tc.tile_pool(name="ops", bufs=4, space="PSUM") as out_ps_pool,
            tc.tile_pool(name="stage", bufs=4) as stage_pool,
        ):
            # ---------- constants & weights (two blob DMAs) ----------
            wb = singles.tile([128, 7 * 128], F16, tag="wb")
            nc.sync.dma_start(out=wb, in_=wb_d[:, :])
            gvt0 = wb[:, 0:128]
            gvt1 = wb[:, 128:256]
            wvt0 = wb[:, 256:384]
            wvt1 = wb[:, 384:512]
            wivt_sb = wb[:, 512:640]
            woutt_sb = wb[:, 640:768]
            bout_sb = wb[0:1, 768:896]
            mb = singles.tile([128, 2 + 2 * 128], F32, tag="mb")
            nc.sync.dma_start(out=mb, in_=mb_d[:, :])
            bv_col = mb[:, 0:1]
            biv_col = mb[:, 1:2]
            gamma_gr = mb[:, 2:130]
            beta_gr = mb[:, 130:258]

            ones32 = singles.tile([1, 128], F32, tag="ones32")
            nc.vector.memset(ones32, 1.0)
            ones16 = singles.tile([1, 128], F16, tag="ones16")
            nc.vector.memset(ones16, 1.0)
            ones8 = singles.tile([1, 2 * 128], F8, tag="ones8")
            nc.vector.memset(ones8, 1.0)
            eps_sb = singles.tile([128, 1], F32, tag="eps")
            nc.vector.memset(eps_sb, EPS)

            iota_i = singles.tile([128, 1], mybir.dt.int32, tag="iota_i")
            nc.gpsimd.iota(iota_i, [[0, 1]], base=0, channel_multiplier=1)
            iota_f = singles.tile([128, 1], F32, tag="iota_f")
            nc.vector.tensor_copy(out=iota_f, in_=iota_i)

            # idx arrives in 8 growing "era" chunks onto partition 0 (a
            # single [1, N] row load would serialize ~38 us on one SBUF
            # partition's write line; small per-group chunk DMAs starve the
            # broadcast on DGE latency). First era is ready in ~1 us; each
            # era completes well before its groups are consumed.
            ERAS = [2, 2, 4, 8, 16, 22, 22, 22]
            era_tiles = []
            era_start = []
            g0 = 0
            for e, ne in enumerate(ERAS):
                et = singles.tile([1, ne * GROUP], F16, tag=f"era{e}")
                src = bass.AP(
                    tensor=idx_d[:, :].tensor, offset=g0 * GROUP,
                    ap=[[0, 1], [1, ne * GROUP]],
                )
                nc.sync.dma_start(out=et, in_=src)
                era_tiles.append(et)
                era_start.append(g0)
                g0 += ne
            assert g0 == NGROUPS

            def idx_slice(g):
                e = 0
                while e + 1 < len(ERAS) and era_start[e + 1] <= g:
                    e += 1
                off = (g - era_start[e]) * GROUP
                return era_tiles[e][:, off:off + GROUP]

            # ---------- table prologue ----------
            # PSUM borrowed from the out pool (same tag -> same ring slots).
            # v.T[h, b] = sum_g WvT[g, h] * gvT[g, b] + bv[h]
            vt_ps = out_ps_pool.tile([128, GROUP], F32, tag="outps")
            nc.tensor.matmul(vt_ps[:, 0:128], wvt0, gvt0, start=True, stop=False)
            nc.tensor.matmul(vt_ps[:, 0:128], wvt1, gvt1, start=False, stop=True)
            vt_sb = singles.tile([128, 128], F16, tag="vt_sb")
            nc.scalar.activation(
                vt_sb, vt_ps[:, 0:128],
                mybir.ActivationFunctionType.Identity, bias=bv_col, scale=1.0,
            )

            # v2.T[j, b] = sum_h WivT[h, j] * vT[h, b] + biv[j]
            v2t_ps = out_ps_pool.tile([128, GROUP], F32, tag="outps")
            nc.tensor.matmul(v2t_ps[:, 0:128], wivt_sb, vt_sb, start=True, stop=True)
            v2t_sb = singles.tile([128, 128], F16, tag="v2t_sb")
            nc.scalar.activation(
                v2t_sb, v2t_ps[:, 0:128],
                mybir.ActivationFunctionType.Identity, bias=biv_col, scale=1.0,
            )

            # ao[b, h2] = sum_j v2T[j, b] * WoutT[j, h2] + bout[h2]
            ao_ps = out_ps_pool.tile([128, GROUP], F32, tag="outps")
            nc.tensor.matmul(ao_ps[:, 0:128], v2t_sb, woutt_sb, start=True, stop=False)
            nc.tensor.matmul(ao_ps[:, 0:128], ones16, bout_sb, start=False, stop=True)

            # LayerNorm over free dim
            stats = singles.tile([128, 6], F32, tag="stats")
            nc.vector.bn_stats(out=stats, in_=ao_ps[:, 0:128])
            mv = singles.tile([128, 2], F32, tag="mv")
            nc.vector.bn_aggr(out=mv, in_=stats)
            rstd = singles.tile([128, 1], F32, tag="rstd")
            nc.scalar.activation(
                rstd, mv[:, 1:2], mybir.ActivationFunctionType.Sqrt,
                bias=eps_sb, scale=1.0,
            )
            nc.vector.reciprocal(out=rstd, in_=rstd)

            tbl = singles.tile([128, 128], F32, tag="tbl")
            nc.vector.tensor_scalar(
                out=tbl, in0=ao_ps[:, 0:128],
                scalar1=mv[:, 0:1], scalar2=rstd,
                op0=mybir.AluOpType.subtract, op1=mybir.AluOpType.mult,
            )
            tbl2 = singles.tile([128, 128], F32, tag="tbl2")
            nc.vector.tensor_mul(out=tbl2, in0=tbl, in1=gamma_gr)
            tbl3 = singles.tile([128, 128], F32, tag="tbl3")
            nc.vector.tensor_add(out=tbl3, in0=tbl2, in1=beta_gr)
            tbl_h = singles.tile([128, 128], F16, tag="tbl_h")
            nc.vector.tensor_copy(out=tbl_h, in_=tbl3)

            ones8_3d = ones8.rearrange("p (j n) -> p j n", j=2)

            # ---------- main gather loop: one group = 512 nodes ----------
            # Store schedule: small head stores ramp the DMA engines while
            # the pipeline fills; a small tail store shortens the drain.
            STORES = [1, 1, 2] + [STORE_G] * 23 + [2]
            assert sum(STORES) == NGROUPS
            sg0 = 0
            for s, gs in enumerate(STORES):
                stage = stage_pool.tile([128, STORE_G * GROUP], F32, tag="stage")
                for gi in range(gs):
                    g = sg0 + gi
                    use_pool = (
                        pool_mod is not None and g % pool_mod == pool_mod - 1
                    )
                    idxc = idx_slice(g)
                    if use_pool:
                        bc_sb = bcsb_pool.tile([128, GROUP], F16, tag="bcsb")
                        nc.gpsimd.partition_broadcast(bc_sb, idxc)
                        eq_in = bc_sb
                    else:
                        bc_ps = bc_ps_pool.tile([128, GROUP], F32, tag="bcast")
                        nc.tensor.matmul(
                            bc_ps, ones16, idxc, start=True, stop=True
                        )
                        eq_in = bc_ps
                    # onehotT[j, p] = (idx[p] == j)
                    oh = oh_pool.tile([128, GROUP], F16, tag="oh")
                    nc.vector.tensor_scalar(
                        out=oh, in0=eq_in,
                        scalar1=iota_f, scalar2=None,
                        op0=mybir.AluOpType.is_equal,
                    )
                    # out[p, h] = sum_j onehotT[j, p] * table[j, h]
                    out_ps = out_ps_pool.tile([128, GROUP], F32, tag="outps")
                    for t in range(GROUP // 128):
                        sl = slice(t * 128, (t + 1) * 128)
                        nc.tensor.matmul(
                            out_ps[:, sl], oh[:, sl], tbl_h,
                            start=True, stop=True,
                        )
                    dst = stage[:, gi * GROUP:(gi + 1) * GROUP]
                    if copy_sched[g % len(copy_sched)] == "A":
                        nc.scalar.copy(out=dst, in_=out_ps)
                    else:
                        nc.vector.tensor_copy(out=dst, in_=out_ps)
                # Node order is host-permuted so partition p owns DRAM rows
                # [p*NT, (p+1)*NT): every store descriptor is a contiguous
                # run per partition (full DMA line rate).
                ts = gs * GROUP // 128                # tiles in this store
                t0 = sg0 * (GROUP // 128)
                dview = out_d[:, :].rearrange("(p t) c -> p t c", p=128)[
                    :, t0:t0 + ts, :
                ]
                sview = stage[:, :gs * GROUP].rearrange("p (t c) -> p t c", c=128)
                # Alternate store triggers between the Sync and Scalar
                # queues: on the Scalar queue the producing copies precede
                # the trigger in-order, so it fires wait-free, and the DGE
                # config of adjacent stores overlaps (better engine packing).
                if s % 2:
                    nc.scalar.dma_start(out=dview, in_=sview)
                else:
                    nc.sync.dma_start(out=dview, in_=sview)
                sg0 += gs

    nc.finalize()
    return nc


_CACHE = {}


def _get_nc(variant=None):
    key = variant or DEFAULT_VARIANT
    if key not in _CACHE:
        _CACHE[key] = build_bass(variant=key)
    return _CACHE[key]


def _prep_in_maps(inputs):
    f32c = lambda x: np.ascontiguousarray(np.asarray(x), dtype=np.float32)
    win = np.asarray(inputs["Win"], dtype=np.float32)
    bin_ = np.asarray(inputs["bin"], dtype=np.float32)
    gvT = np.asarray(inputs["graph_vec"], dtype=np.float32).T
    WvT = np.asarray(inputs["Wv"], dtype=np.float32).T
    WivT = win[2 * H:3 * H, :].T
    WoutT = np.asarray(inputs["Wout"], dtype=np.float32).T
    blob = np.zeros((128, 7 * 128), dtype=np.float16)
    blob[:, 0:128] = gvT[0:128]
    blob[:, 128:256] = gvT[128:256]
    blob[:, 256:384] = WvT[0:128]
    blob[:, 384:512] = WvT[128:256]
    blob[:, 512:640] = WivT
    blob[:, 640:768] = WoutT
    blob[0, 768:896] = np.asarray(inputs["bout"], dtype=np.float16)
    mblob = np.zeros((128, 2 + 2 * 128), dtype=np.float32)
    mblob[:, 0] = f32c(inputs["bv"])
    mblob[:, 1] = f32c(bin_[2 * H:3 * H])
    mblob[:, 2:130] = np.tile(f32c(inputs["gamma"])[None, :], (128, 1))
    mblob[:, 130:258] = np.tile(f32c(inputs["beta"])[None, :], (128, 1))
    shared = {
        "wblob": np.ascontiguousarray(blob),
        "mblob": np.ascontiguousarray(mblob),
    }
    bi = np.asarray(inputs["batch_indices"]).astype(np.int64).reshape(N_CORES, NSHARD)
    idx_pad = np.zeros((N_CORES, NPAD), dtype=np.int64)
    idx_pad[:, :NSHARD] = bi
    # Permute so device tile t covers nodes {p*NT + t}: partition p then owns
    # the contiguous output-row block [p*NT, (p+1)*NT) (contiguous DMA runs).
    nt = NPAD // 128
    idx_tr = idx_pad.reshape(N_CORES, 128, nt).transpose(0, 2, 1)  # [c, t, p]
    idx_flat = idx_tr.reshape(N_CORES, NGROUPS, GROUP)
    idx_f16 = idx_flat.astype(np.float16)  # exact: values < 2048
    return [
        {**shared, "idx": np.ascontiguousarray(idx_f16[c])}
        for c in range(N_CORES)
    ]


def run_sharded(inputs, trace=False, variant=None, **kwargs):
    """Run the SPMD bass kernel on 8 cores; returns (output, BassKernelResults)."""
    kwargs.pop("precision", None)  # legacy knob
    in_maps = _prep_in_maps(inputs)
    nc = _get_nc(variant)
    res = bass_utils.run_bass_kernel_spmd(
        nc, in_maps, core_ids=list(range(N_CORES)), trace=trace, **kwargs
    )
    shards = [r["out"][:NSHARD] for r in res.results]
    out = np.concatenate(shards, axis=0)
    return out, res


def kernel(**inputs) -> np.ndarray:
    out, _ = run_sharded(inputs)
    return out


# revision 24
# speedup vs baseline: 1.0296x; 1.0296x over previous
"""Trainium2 Bass kernel for nn_CrossAttentionBlock.

Math: with key/value seq_len == 1 the attention softmax is identically 1, so
q/k (and masked_x entirely) never affect the output:

    out[n, :] = LN(((graph_vec @ Wv.T + bv) @ Wiv.T + biv) @ Wout.T + bout)[batch_indices[n]]

i.e. a 128-row lookup table indexed by batch_indices. Strategy per core
(data-parallel over nodes, 8 cores x 50000 nodes):

  1. prologue: compute the [128, 128] table on-device from host-transposed
     weight layouts (3 matmuls + bias rank-1 updates + bn_stats LayerNorm),
     cast to fp16 (rel err ~5e-4, far under the 2e-2 gate).
  2. main loop per 512-node group (one PSUM bank):
       - idx chunk DMA (1 KB fp8) onto partition 0, ring of 12, Pool-queue
         trigger (a single [1, N] idx load would serialize ~38 us on one
         SBUF partition's write port)
       - broadcast idx across partitions with ONE PE DoubleRow fp8 matmul:
         idx = 16q + r split host-side, both fp8-exact; ones (x) [q; r]
         accumulates q+r = idx into PSUM at 0.5 cycles/column
       - DVE is_equal against a partition-iota column -> one-hot^T (fp16)
       - 4x PE fp16 matmuls: out[node, h] = onehotT.T @ tbl
       - PSUM -> SBUF staging copy (Scalar; every 6th on DVE)
       - one 512 KiB DMA store per 2 groups

The only irreducible HBM traffic is the 25.7 MiB/core fp32 output write
(~75 us at ~350 GB/s); every other engine is scheduled to stay under that.
"""

import sys

if "/opt/trn_rl_repo" not in sys.path:
    sys.path.insert(0, "/opt/trn_rl_repo")

import numpy as np

import concourse.bass as bass
import concourse.bacc as bacc
import concourse.tile as tile
from concourse import mybir
from concourse import bass_utils

F32 = mybir.dt.float32
F16 = mybir.dt.float16
F8 = mybir.dt.float8e4
DR = mybir.MatmulPerfMode.DoubleRow

N_NODES = 400000
H = 128          # hidden
G = 256          # graph_dim
B = 128          # batch (table rows)
N_CORES = 8
NSHARD = N_NODES // N_CORES          # 50000
GROUP = 512                          # nodes per group (one PSUM bank)
NPAD = 50176                         # 98 * 512, per-core padded shard
NGROUPS = NPAD // GROUP              # 98
STORE_G = 4                          # groups per DMA store (1 MiB)
EPS = 1e-5

# Schedule knobs (variant tuple: (dr_bcast, copy_sched, pool_mod))
#   dr_bcast: broadcast via one fp8 DoubleRow matmul (else fp16 K=1 matmul)
#   copy_sched: string over {"A","V"} cycled per group for the PSUM->stage
#               copy engine (Scalar/DVE; GpSimd cannot read PSUM)
#   pool_mod: m -> groups with g % m == m-1 broadcast on GpSimd
#             partition_broadcast (None = all broadcasts on PE)
DEFAULT_VARIANT = (False, "AAAAAV", 2)


def _row1(ap):
    """View a 1-D DRAM AP as [1, N]."""
    return bass.AP(tensor=ap.tensor, offset=ap.offset, ap=[[0, 1]] + list(ap.ap))


def _bcast128(ap):
    """View a 1-D DRAM AP as [128, N] replicated across partitions."""
    return bass.AP(tensor=ap.tensor, offset=ap.offset, ap=[[0, 128]] + list(ap.ap))


def build_bass(variant=DEFAULT_VARIANT):
    dr_bcast, copy_sched, pool_mod = variant
    nc = bacc.Bacc("TRN2", target_bir_lowering=False)

    # packed host-transposed weights (two DMAs):
    # fp16: [gvT0|gvT1|WvT0|WvT1|WivT|WoutT|bout_row@p0]
    # fp32: [bv_col|biv_col|gamma replicated|beta replicated]
    wb_d = nc.dram_tensor("wblob", [128, 7 * 128], F16, kind="ExternalInput")
    mb_d = nc.dram_tensor("mblob", [128, 2 + 2 * 128], F32, kind="ExternalInput")
    idx_d = nc.dram_tensor("idx", [NGROUPS, GROUP], F16, kind="ExternalInput")
    out_d = nc.dram_tensor("out", [NPAD, H], F32, kind="ExternalOutput")

    with tile.TileContext(nc) as tc:
        with (
            tc.tile_pool(name="singles", bufs=1) as singles,
            tc.tile_pool(name="oh", bufs=6) as oh_pool,
            tc.tile_pool(name="bcsb", bufs=3) as bcsb_pool,
            tc.tile_pool(name="bps", bufs=3, space="PSUM") as bc_ps_pool,
            tc.tile_pool(name="ops", bufs=4, space="PSUM") as out_ps_pool,
            tc.tile_pool(name="stage", bufs=4) as stage_pool,
        ):
            # ---------- constants & weights (two blob DMAs) ----------
            wb = singles.tile([128, 7 * 128], F16, tag="wb")
            nc.sync.dma_start(out=wb, in_=wb_d[:, :])
            gvt0 = wb[:, 0:128]
            gvt1 = wb[:, 128:256]
            wvt0 = wb[:, 256:384]
            wvt1 = wb[:, 384:512]
            wivt_sb = wb[:, 512:640]
            woutt_sb = wb[:, 640:768]
            bout_sb = wb[0:1, 768:896]
            mb = singles.tile([128, 2 + 2 * 128], F32, tag="mb")
            nc.sync.dma_start(out=mb, in_=mb_d[:, :])
            bv_col = mb[:, 0:1]
            biv_col = mb[:, 1:2]
            gamma_gr = mb[:, 2:130]
            beta_gr = mb[:, 130:258]

            ones32 = singles.tile([1, 128], F32, tag="ones32")
            nc.vector.memset(ones32, 1.0)
            ones16 = singles.tile([1, 128], F16, tag="ones16")
            nc.vector.memset(ones16, 1.0)
            ones8 = singles.tile([1, 2 * 128], F8, tag="ones8")
            nc.vector.memset(ones8, 1.0)
            eps_sb = singles.tile([128, 1], F32, tag="eps")
            nc.vector.memset(eps_sb, EPS)

            iota_i = singles.tile([128, 1], mybir.dt.int32, tag="iota_i")
            nc.gpsimd.iota(iota_i, [[0, 1]], base=0, channel_multiplier=1)
            iota_f = singles.tile([128, 1], F32, tag="iota_f")
            nc.vector.tensor_copy(out=iota_f, in_=iota_i)

            # idx arrives in 8 growing "era" chunks onto partition 0 (a
            # single [1, N] row load would serialize ~38 us on one SBUF
            # partition's write line; small per-group chunk DMAs starve the
            # broadcast on DGE latency). First era is ready in ~1 us; each
            # era completes well before its groups are consumed.
            ERAS = [2, 2, 4, 8, 16, 22, 22, 22]
            era_tiles = []
            era_start = []
            g0 = 0
            for e, ne in enumerate(ERAS):
                et = singles.tile([1, ne * GROUP], F16, tag=f"era{e}")
                src = bass.AP(
                    tensor=idx_d[:, :].tensor, offset=g0 * GROUP,
                    ap=[[0, 1], [1, ne * GROUP]],
                )
                nc.sync.dma_start(out=et, in_=src)
                era_tiles.append(et)
                era_start.append(g0)
                g0 += ne
            assert g0 == NGROUPS

            def idx_slice(g):
                e = 0
                while e + 1 < len(ERAS) and era_start[e + 1] <= g:
                    e += 1
                off = (g - era_start[e]) * GROUP
                return era_tiles[e][:, off:off + GROUP]

            # ---------- table prologue ----------
            # PSUM borrowed from the out pool (same tag -> same ring slots).
            # v.T[h, b] = sum_g WvT[g, h] * gvT[g, b] + bv[h]
            vt_ps = out_ps_pool.tile([128, GROUP], F32, tag="outps")
            nc.tensor.matmul(vt_ps[:, 0:128], wvt0, gvt0, start=True, stop=False)
            nc.tensor.matmul(vt_ps[:, 0:128], wvt1, gvt1, start=False, stop=True)
            vt_sb = singles.tile([128, 128], F16, tag="vt_sb")
            nc.scalar.activation(
                vt_sb, vt_ps[:, 0:128],
                mybir.ActivationFunctionType.Identity, bias=bv_col, scale=1.0,
            )

            # v2.T[j, b] = sum_h WivT[h, j] * vT[h, b] + biv[j]
            v2t_ps = out_ps_pool.tile([128, GROUP], F32, tag="outps")
            nc.tensor.matmul(v2t_ps[:, 0:128], wivt_sb, vt_sb, start=True, stop=True)
            v2t_sb = singles.tile([128, 128], F16, tag="v2t_sb")
            nc.scalar.activation(
                v2t_sb, v2t_ps[:, 0:128],
                mybir.ActivationFunctionType.Identity, bias=biv_col, scale=1.0,
            )

            # ao[b, h2] = sum_j v2T[j, b] * WoutT[j, h2] + bout[h2]
            ao_ps = out_ps_pool.tile([128, GROUP], F32, tag="outps")
            nc.tensor.matmul(ao_ps[:, 0:128], v2t_sb, woutt_sb, start=True, stop=False)
            nc.tensor.matmul(ao_ps[:, 0:128], ones16, bout_sb, start=False, stop=True)

            # LayerNorm over free dim
            stats = singles.tile([128, 6], F32, tag="stats")
            nc.vector.bn_stats(out=stats, in_=ao_ps[:, 0:128])
            mv = singles.tile([128, 2], F32, tag="mv")
            nc.vector.bn_aggr(out=mv, in_=stats)
            rstd = singles.tile([128, 1], F32, tag="rstd")
            nc.scalar.activation(
                rstd, mv[:, 1:2], mybir.ActivationFunctionType.Sqrt,
                bias=eps_sb, scale=1.0,
            )
            nc.vector.reciprocal(out=rstd, in_=rstd)

            tbl = singles.tile([128, 128], F32, tag="tbl")
            nc.vector.tensor_scalar(
                out=tbl, in0=ao_ps[:, 0:128],
                scalar1=mv[:, 0:1], scalar2=rstd,
                op0=mybir.AluOpType.subtract, op1=mybir.AluOpType.mult,
            )
            tbl2 = singles.tile([128, 128], F32, tag="tbl2")
            nc.vector.tensor_mul(out=tbl2, in0=tbl, in1=gamma_gr)
            tbl3 = singles.tile([128, 128], F32, tag="tbl3")
            nc.vector.tensor_add(out=tbl3, in0=tbl2, in1=beta_gr)
            tbl_h = singles.tile([128, 128], F16, tag="tbl_h")
            nc.vector.tensor_copy(out=tbl_h, in_=tbl3)

            ones8_3d = ones8.rearrange("p (j n) -> p j n", j=2)

            # ---------- main gather loop: one group = 512 nodes ----------
            # Store schedule: small head stores ramp the DMA engines while
            # the pipeline fills; a small tail store shortens the drain.
            STORES = [1, 1, 2] + [STORE_G] * 23 + [2]
            assert sum(STORES) == NGROUPS
            sg0 = 0
            for s, gs in enumerate(STORES):
                stage = stage_pool.tile([128, STORE_G * GROUP], F32, tag="stage")
                for gi in range(gs):
                    g = sg0 + gi
                    use_pool = (
                        pool_mod is not None and g % pool_mod == pool_mod - 1
                    )
                    idxc = idx_slice(g)
                    if use_pool:
                        bc_sb = bcsb_pool.tile([128, GROUP], F16, tag="bcsb")
                        nc.gpsimd.partition_broadcast(bc_sb, idxc)
                        eq_in = bc_sb
                    else:
                        bc_ps = bc_ps_pool.tile([128, GROUP], F32, tag="bcast")
                        nc.tensor.matmul(
                            bc_ps, ones16, idxc, start=True, stop=True
                        )
                        eq_in = bc_ps
                    # onehotT[j, p] = (idx[p] == j)
                    oh = oh_pool.tile([128, GROUP], F16, tag="oh")
                    nc.vector.tensor_scalar(
                        out=oh, in0=eq_in,
                        scalar1=iota_f, scalar2=None,
                        op0=mybir.AluOpType.is_equal,
                    )
                    # out[p, h] = sum_j onehotT[j, p] * table[j, h]
                    out_ps = out_ps_pool.tile([128, GROUP], F32, tag="outps")
                    for t in range(GROUP // 128):
                        sl = slice(t * 128, (t + 1) * 128)
                        nc.tensor.matmul(
                            out_ps[:, sl], oh[:, sl], tbl_h,
                            start=True, stop=True,
                        )
                    dst = stage[:, gi * GROUP:(gi + 1) * GROUP]
                    if copy_sched[g % len(copy_sched)] == "A":
                        nc.scalar.copy(out=dst, in_=out_ps)
                    else:
                        nc.vector.tensor_copy(out=dst, in_=out_ps)
                # Node order is host-permuted so partition p owns DRAM rows
                # [p*NT, (p+1)*NT): every store descriptor is a contiguous
                # run per partition (full DMA line rate).
                ts = gs * GROUP // 128                # tiles in this store
                t0 = sg0 * (GROUP // 128)
                dview = out_d[:, :].rearrange("(p t) c -> p t c", p=128)[
                    :, t0:t0 + ts, :
                ]
                sview = stage[:, :gs * GROUP].rearrange("p (t c) -> p t c", c=128)
                nc.sync.dma_start(out=dview, in_=sview)
                sg0 += gs

    nc.finalize()
    return nc


_CACHE = {}


def _get_nc(variant=None):
    key = variant or DEFAULT_VARIANT
    if key not in _CACHE:
        _CACHE[key] = build_bass(variant=key)
    return _CACHE[key]


def _prep_in_maps(inputs):
    f32c = lambda x: np.ascontiguousarray(np.asarray(x), dtype=np.float32)
    win = np.asarray(inputs["Win"], dtype=np.float32)
    bin_ = np.asarray(inputs["bin"], dtype=np.float32)
    gvT = np.asarray(inputs["graph_vec"], dtype=np.float32).T
    WvT = np.asarray(inputs["Wv"], dtype=np.float32).T
    WivT = win[2 * H:3 * H, :].T
    WoutT = np.asarray(inputs["Wout"], dtype=np.float32).T
    blob = np.zeros((128, 7 * 128), dtype=np.float16)
    blob[:, 0:128] = gvT[0:128]
    blob[:, 128:256] = gvT[128:256]
    blob[:, 256:384] = WvT[0:128]
    blob[:, 384:512] = WvT[128:256]
    blob[:, 512:640] = WivT
    blob[:, 640:768] = WoutT
    blob[0, 768:896] = np.asarray(inputs["bout"], dtype=np.float16)
    mblob = np.zeros((128, 2 + 2 * 128), dtype=np.float32)
    mblob[:, 0] = f32c(inputs["bv"])
    mblob[:, 1] = f32c(bin_[2 * H:3 * H])
    mblob[:, 2:130] = np.tile(f32c(inputs["gamma"])[None, :], (128, 1))
    mblob[:, 130:258] = np.tile(f32c(inputs["beta"])[None, :], (128, 1))
    shared = {
        "wblob": np.ascontiguousarray(blob),
        "mblob": np.ascontiguousarray(mblob),
    }
    bi = np.asarray(inputs["batch_indices"]).astype(np.int64).reshape(N_CORES, NSHARD)
    idx_pad = np.zeros((N_CORES, NPAD), dtype=np.int64)
    idx_pad[:, :NSHARD] = bi
    # Permute so device tile t covers nodes {p*NT + t}: partition p then owns
    # the contiguous output-row block [p*NT, (p+1)*NT) (contiguous DMA runs).
    nt = NPAD // 128
    idx_tr = idx_pad.reshape(N_CORES, 128, nt).transpose(0, 2, 1)  # [c, t, p]
    idx_flat = idx_tr.reshape(N_CORES, NGROUPS, GROUP)
    idx_f16 = idx_flat.astype(np.float16)  # exact: values < 2048
    return [
        {**shared, "idx": np.ascontiguousarray(idx_f16[c])}
        for c in range(N_CORES)
    ]


def run_sharded(inputs, trace=False, variant=None, **kwargs):
    """Run the SPMD bass kernel on 8 cores; returns (output, BassKernelResults)."""
    kwargs.pop("precision", None)  # legacy knob
    in_maps = _prep_in_maps(inputs)
    nc = _get_nc(variant)
    res = bass_utils.run_bass_kernel_spmd(
        nc, in_maps, core_ids=list(range(N_CORES)), trace=trace, **kwargs
    )
    shards = [r["out"][:NSHARD] for r in res.results]
    out = np.concatenate(shards, axis=0)
    return out, res


def kernel(**inputs) -> np.ndarray:
    out, _ = run_sharded(inputs)
    return out


# revision 25
# speedup vs baseline: 1.0471x; 1.0169x over previous
"""Trainium2 Bass kernel for nn_CrossAttentionBlock.

Math: with key/value seq_len == 1 the attention softmax is identically 1, so
q/k (and masked_x entirely) never affect the output:

    out[n, :] = LN(((graph_vec @ Wv.T + bv) @ Wiv.T + biv) @ Wout.T + bout)[batch_indices[n]]

i.e. a 128-row lookup table indexed by batch_indices. Strategy per core
(data-parallel over nodes, 8 cores x 50000 nodes):

  1. prologue: compute the [128, 128] table on-device from host-transposed
     weight layouts (3 matmuls + bias rank-1 updates + bn_stats LayerNorm),
     cast to fp16 (rel err ~5e-4, far under the 2e-2 gate).
  2. main loop per 512-node group (one PSUM bank):
       - idx chunk DMA (1 KB fp8) onto partition 0, ring of 12, Pool-queue
         trigger (a single [1, N] idx load would serialize ~38 us on one
         SBUF partition's write port)
       - broadcast idx across partitions with ONE PE DoubleRow fp8 matmul:
         idx = 16q + r split host-side, both fp8-exact; ones (x) [q; r]
         accumulates q+r = idx into PSUM at 0.5 cycles/column
       - DVE is_equal against a partition-iota column -> one-hot^T (fp16)
       - 4x PE fp16 matmuls: out[node, h] = onehotT.T @ tbl
       - PSUM -> SBUF staging copy (Scalar; every 6th on DVE)
       - one 512 KiB DMA store per 2 groups

The only irreducible HBM traffic is the 25.7 MiB/core fp32 output write
(~75 us at ~350 GB/s); every other engine is scheduled to stay under that.
"""

import sys

if "/opt/trn_rl_repo" not in sys.path:
    sys.path.insert(0, "/opt/trn_rl_repo")

import numpy as np

import concourse.bass as bass
import concourse.bacc as bacc
import concourse.tile as tile
from concourse import mybir
from concourse import bass_utils

F32 = mybir.dt.float32
F16 = mybir.dt.float16
F8 = mybir.dt.float8e4
DR = mybir.MatmulPerfMode.DoubleRow

N_NODES = 400000
H = 128          # hidden
G = 256          # graph_dim
B = 128          # batch (table rows)
N_CORES = 8
NSHARD = N_NODES // N_CORES          # 50000
GROUP = 512                          # nodes per group (one PSUM bank)
NPAD = 50176                         # 98 * 512, per-core padded shard
NGROUPS = NPAD // GROUP              # 98
STORE_G = 4                          # groups per DMA store (1 MiB)
EPS = 1e-5

# Schedule knobs (variant tuple: (dr_bcast, copy_sched, pool_mod))
#   dr_bcast: broadcast via one fp8 DoubleRow matmul (else fp16 K=1 matmul)
#   copy_sched: string over {"A","V"} cycled per group for the PSUM->stage
#               copy engine (Scalar/DVE; GpSimd cannot read PSUM)
#   pool_mod: m -> groups with g % m == m-1 broadcast on GpSimd
#             partition_broadcast (None = all broadcasts on PE)
DEFAULT_VARIANT = (False, "AAAAAV", 2)


def _row1(ap):
    """View a 1-D DRAM AP as [1, N]."""
    return bass.AP(tensor=ap.tensor, offset=ap.offset, ap=[[0, 1]] + list(ap.ap))


def _bcast128(ap):
    """View a 1-D DRAM AP as [128, N] replicated across partitions."""
    return bass.AP(tensor=ap.tensor, offset=ap.offset, ap=[[0, 128]] + list(ap.ap))


def build_bass(variant=DEFAULT_VARIANT):
    dr_bcast, copy_sched, pool_mod = variant
    nc = bacc.Bacc("TRN2", target_bir_lowering=False)

    # packed host-transposed weights (two DMAs):
    # fp16: [gvT0|gvT1|WvT0|WvT1|WivT|WoutT|bout_row@p0]
    # fp32: [bv_col|biv_col|gamma replicated|beta replicated]
    wb_d = nc.dram_tensor("wblob", [128, 7 * 128], F16, kind="ExternalInput")
    mb_d = nc.dram_tensor("mblob", [128, 2 + 2 * 128], F32, kind="ExternalInput")
    idx_d = nc.dram_tensor("idx", [NGROUPS, GROUP], F16, kind="ExternalInput")
    out_d = nc.dram_tensor("out", [NPAD, H], F32, kind="ExternalOutput")

    with tile.TileContext(nc) as tc:
        with (
            tc.tile_pool(name="singles", bufs=1) as singles,
            tc.tile_pool(name="oh", bufs=8) as oh_pool,
            tc.tile_pool(name="bcsb", bufs=4) as bcsb_pool,
            tc.tile_pool(name="bps", bufs=3, space="PSUM") as bc_ps_pool,
            tc.tile_pool(name="ops", bufs=4, space="PSUM") as out_ps_pool,
            tc.tile_pool(name="stage", bufs=4) as stage_pool,
        ):
            # ---------- constants & weights (two blob DMAs) ----------
            wb = singles.tile([128, 7 * 128], F16, tag="wb")
            nc.sync.dma_start(out=wb, in_=wb_d[:, :])
            gvt0 = wb[:, 0:128]
            gvt1 = wb[:, 128:256]
            wvt0 = wb[:, 256:384]
            wvt1 = wb[:, 384:512]
            wivt_sb = wb[:, 512:640]
            woutt_sb = wb[:, 640:768]
            bout_sb = wb[0:1, 768:896]
            mb = singles.tile([128, 2 + 2 * 128], F32, tag="mb")
            nc.sync.dma_start(out=mb, in_=mb_d[:, :])
            bv_col = mb[:, 0:1]
            biv_col = mb[:, 1:2]
            gamma_gr = mb[:, 2:130]
            beta_gr = mb[:, 130:258]

            ones32 = singles.tile([1, 128], F32, tag="ones32")
            nc.vector.memset(ones32, 1.0)
            ones16 = singles.tile([1, 128], F16, tag="ones16")
            nc.vector.memset(ones16, 1.0)
            ones8 = singles.tile([1, 2 * 128], F8, tag="ones8")
            nc.vector.memset(ones8, 1.0)
            eps_sb = singles.tile([128, 1], F32, tag="eps")
            nc.vector.memset(eps_sb, EPS)

            iota_i = singles.tile([128, 1], mybir.dt.int32, tag="iota_i")
            nc.gpsimd.iota(iota_i, [[0, 1]], base=0, channel_multiplier=1)
            iota_f = singles.tile([128, 1], F32, tag="iota_f")
            nc.vector.tensor_copy(out=iota_f, in_=iota_i)
            # Warm the Scalar activation table (Copy/Sqrt set) while the
            # weight blobs are still in flight: the first real ACTIVATE
            # otherwise eats a 1.3 us ACT_TABLE_LOAD on the critical path.
            warm = singles.tile([128, 1], F32, tag="warm")
            nc.scalar.copy(out=warm, in_=eps_sb)
            nc.scalar.activation(
                warm, eps_sb, mybir.ActivationFunctionType.Sqrt,
                bias=eps_sb, scale=1.0,
            )

            # idx arrives in 8 growing "era" chunks onto partition 0 (a
            # single [1, N] row load would serialize ~38 us on one SBUF
            # partition's write line; small per-group chunk DMAs starve the
            # broadcast on DGE latency). First era is ready in ~1 us; each
            # era completes well before its groups are consumed.
            ERAS = [2, 2, 4, 8, 16, 22, 22, 22]
            era_tiles = []
            era_start = []
            g0 = 0
            for e, ne in enumerate(ERAS):
                et = singles.tile([1, ne * GROUP], F16, tag=f"era{e}")
                src = bass.AP(
                    tensor=idx_d[:, :].tensor, offset=g0 * GROUP,
                    ap=[[0, 1], [1, ne * GROUP]],
                )
                nc.sync.dma_start(out=et, in_=src)
                era_tiles.append(et)
                era_start.append(g0)
                g0 += ne
            assert g0 == NGROUPS

            def idx_slice(g):
                e = 0
                while e + 1 < len(ERAS) and era_start[e + 1] <= g:
                    e += 1
                off = (g - era_start[e]) * GROUP
                return era_tiles[e][:, off:off + GROUP]

            # ---------- table prologue ----------
            # PSUM borrowed from the out pool (same tag -> same ring slots).
            # v.T[h, b] = sum_g WvT[g, h] * gvT[g, b] + bv[h]
            vt_ps = out_ps_pool.tile([128, GROUP], F32, tag="outps")
            nc.tensor.matmul(vt_ps[:, 0:128], wvt0, gvt0, start=True, stop=False)
            nc.tensor.matmul(vt_ps[:, 0:128], wvt1, gvt1, start=False, stop=True)
            vt_sb = singles.tile([128, 128], F16, tag="vt_sb")
            nc.scalar.activation(
                vt_sb, vt_ps[:, 0:128],
                mybir.ActivationFunctionType.Identity, bias=bv_col, scale=1.0,
            )

            # v2.T[j, b] = sum_h WivT[h, j] * vT[h, b] + biv[j]
            v2t_ps = out_ps_pool.tile([128, GROUP], F32, tag="outps")
            nc.tensor.matmul(v2t_ps[:, 0:128], wivt_sb, vt_sb, start=True, stop=True)
            v2t_sb = singles.tile([128, 128], F16, tag="v2t_sb")
            nc.scalar.activation(
                v2t_sb, v2t_ps[:, 0:128],
                mybir.ActivationFunctionType.Identity, bias=biv_col, scale=1.0,
            )

            # ao[b, h2] = sum_j v2T[j, b] * WoutT[j, h2] + bout[h2]
            ao_ps = out_ps_pool.tile([128, GROUP], F32, tag="outps")
            nc.tensor.matmul(ao_ps[:, 0:128], v2t_sb, woutt_sb, start=True, stop=False)
            nc.tensor.matmul(ao_ps[:, 0:128], ones16, bout_sb, start=False, stop=True)

            # LayerNorm over free dim
            stats = singles.tile([128, 6], F32, tag="stats")
            nc.vector.bn_stats(out=stats, in_=ao_ps[:, 0:128])
            mv = singles.tile([128, 2], F32, tag="mv")
            nc.vector.bn_aggr(out=mv, in_=stats)
            rstd = singles.tile([128, 1], F32, tag="rstd")
            nc.scalar.activation(
                rstd, mv[:, 1:2], mybir.ActivationFunctionType.Sqrt,
                bias=eps_sb, scale=1.0,
            )
            nc.vector.reciprocal(out=rstd, in_=rstd)

            tbl = singles.tile([128, 128], F32, tag="tbl")
            nc.vector.tensor_scalar(
                out=tbl, in0=ao_ps[:, 0:128],
                scalar1=mv[:, 0:1], scalar2=rstd,
                op0=mybir.AluOpType.subtract, op1=mybir.AluOpType.mult,
            )
            tbl2 = singles.tile([128, 128], F32, tag="tbl2")
            nc.vector.tensor_mul(out=tbl2, in0=tbl, in1=gamma_gr)
            tbl3 = singles.tile([128, 128], F32, tag="tbl3")
            nc.vector.tensor_add(out=tbl3, in0=tbl2, in1=beta_gr)
            tbl_h = singles.tile([128, 128], F16, tag="tbl_h")
            nc.vector.tensor_copy(out=tbl_h, in_=tbl3)

            ones8_3d = ones8.rearrange("p (j n) -> p j n", j=2)

            # ---------- main gather loop: one group = 512 nodes ----------
            # Store schedule: small head stores ramp the DMA engines while
            # the pipeline fills; a small tail store shortens the drain.
            STORES = [1, 1, 2] + [STORE_G] * 23 + [2]
            assert sum(STORES) == NGROUPS
            sg0 = 0
            for s, gs in enumerate(STORES):
                stage = stage_pool.tile([128, STORE_G * GROUP], F32, tag="stage")
                for gi in range(gs):
                    g = sg0 + gi
                    use_pool = (
                        pool_mod is not None and g % pool_mod == pool_mod - 1
                    )
                    idxc = idx_slice(g)
                    if use_pool:
                        bc_sb = bcsb_pool.tile([128, GROUP], F16, tag="bcsb")
                        nc.gpsimd.partition_broadcast(bc_sb, idxc)
                        eq_in = bc_sb
                    else:
                        bc_ps = bc_ps_pool.tile([128, GROUP], F32, tag="bcast")
                        nc.tensor.matmul(
                            bc_ps, ones16, idxc, start=True, stop=True
                        )
                        eq_in = bc_ps
                    # onehotT[j, p] = (idx[p] == j)
                    oh = oh_pool.tile([128, GROUP], F16, tag="oh")
                    nc.vector.tensor_scalar(
                        out=oh, in0=eq_in,
                        scalar1=iota_f, scalar2=None,
                        op0=mybir.AluOpType.is_equal,
                    )
                    # out[p, h] = sum_j onehotT[j, p] * table[j, h]
                    out_ps = out_ps_pool.tile([128, GROUP], F32, tag="outps")
                    for t in range(GROUP // 128):
                        sl = slice(t * 128, (t + 1) * 128)
                        nc.tensor.matmul(
                            out_ps[:, sl], oh[:, sl], tbl_h,
                            start=True, stop=True,
                        )
                    dst = stage[:, gi * GROUP:(gi + 1) * GROUP]
                    if copy_sched[g % len(copy_sched)] == "A":
                        nc.scalar.copy(out=dst, in_=out_ps)
                    else:
                        nc.vector.tensor_copy(out=dst, in_=out_ps)
                # Node order is host-permuted so partition p owns DRAM rows
                # [p*NT, (p+1)*NT): every store descriptor is a contiguous
                # run per partition (full DMA line rate).
                ts = gs * GROUP // 128                # tiles in this store
                t0 = sg0 * (GROUP // 128)
                dview = out_d[:, :].rearrange("(p t) c -> p t c", p=128)[
                    :, t0:t0 + ts, :
                ]
                sview = stage[:, :gs * GROUP].rearrange("p (t c) -> p t c", c=128)
                nc.sync.dma_start(out=dview, in_=sview)
                sg0 += gs

    nc.finalize()
    return nc


_CACHE = {}


def _get_nc(variant=None):
    key = variant or DEFAULT_VARIANT
    if key not in _CACHE:
        _CACHE[key] = build_bass(variant=key)
    return _CACHE[key]


def _prep_in_maps(inputs):
    f32c = lambda x: np.ascontiguousarray(np.asarray(x), dtype=np.float32)
    win = np.asarray(inputs["Win"], dtype=np.float32)
    bin_ = np.asarray(inputs["bin"], dtype=np.float32)
    gvT = np.asarray(inputs["graph_vec"], dtype=np.float32).T
    WvT = np.asarray(inputs["Wv"], dtype=np.float32).T
    WivT = win[2 * H:3 * H, :].T
    WoutT = np.asarray(inputs["Wout"], dtype=np.float32).T
    blob = np.zeros((128, 7 * 128), dtype=np.float16)
    blob[:, 0:128] = gvT[0:128]
    blob[:, 128:256] = gvT[128:256]
    blob[:, 256:384] = WvT[0:128]
    blob[:, 384:512] = WvT[128:256]
    blob[:, 512:640] = WivT
    blob[:, 640:768] = WoutT
    blob[0, 768:896] = np.asarray(inputs["bout"], dtype=np.float16)
    mblob = np.zeros((128, 2 + 2 * 128), dtype=np.float32)
    mblob[:, 0] = f32c(inputs["bv"])
    mblob[:, 1] = f32c(bin_[2 * H:3 * H])
    mblob[:, 2:130] = np.tile(f32c(inputs["gamma"])[None, :], (128, 1))
    mblob[:, 130:258] = np.tile(f32c(inputs["beta"])[None, :], (128, 1))
    shared = {
        "wblob": np.ascontiguousarray(blob),
        "mblob": np.ascontiguousarray(mblob),
    }
    bi = np.asarray(inputs["batch_indices"]).astype(np.int64).reshape(N_CORES, NSHARD)
    idx_pad = np.zeros((N_CORES, NPAD), dtype=np.int64)
    idx_pad[:, :NSHARD] = bi
    # Permute so device tile t covers nodes {p*NT + t}: partition p then owns
    # the contiguous output-row block [p*NT, (p+1)*NT) (contiguous DMA runs).
    nt = NPAD // 128
    idx_tr = idx_pad.reshape(N_CORES, 128, nt).transpose(0, 2, 1)  # [c, t, p]
    idx_flat = idx_tr.reshape(N_CORES, NGROUPS, GROUP)
    idx_f16 = idx_flat.astype(np.float16)  # exact: values < 2048
    return [
        {**shared, "idx": np.ascontiguousarray(idx_f16[c])}
        for c in range(N_CORES)
    ]


def run_sharded(inputs, trace=False, variant=None, **kwargs):
    """Run the SPMD bass kernel on 8 cores; returns (output, BassKernelResults)."""
    kwargs.pop("precision", None)  # legacy knob
    in_maps = _prep_in_maps(inputs)
    nc = _get_nc(variant)
    res = bass_utils.run_bass_kernel_spmd(
        nc, in_maps, core_ids=list(range(N_CORES)), trace=trace, **kwargs
    )
    shards = [r["out"][:NSHARD] for r in res.results]
    out = np.concatenate(shards, axis=0)
    return out, res


def kernel(**inputs) -> np.ndarray:
    out, _ = run_sharded(inputs)
    return out
